# revision 2
# baseline (speedup 1.0000x reference)
"""TRN2 Bass kernel for nn_PQLayer (product-quantization soft assignment).

Contract: kernel(x, C) with x [65536, 512] f32, C [8, 256, 64] f32 ->
returns (x_hat [65536, 512] f32, codes [65536, 8, 256] f32), matching

    x_ = x.reshape(B, 8, 64);  xn = x_ / max(||x_||, eps)
    ips = einsum('bmd,mkd->bmk', xn, C)
    codes = softmax(ips, axis=-1)
    x_hat = einsum('mkd,bmk->bmd', C, codes).reshape(B, 512)

Sharding: data-parallel over B across 8 NeuronCores (8192 rows each);
the codebook C is replicated (as two small derived constant tensors).

Per-core layout strategy (all on-device):
  - block = 512 rows (4 sub-tiles of 128); 16 blocks per core.
  - normalize: x^2 (GPSIMD) -> segmented reduce (DVE) -> Ln -> Exp(-0.5)
    on ACT (both functions live in the same activation table set as the
    main Exp, so no table thrashing); xn = x * rsqrt (GPSIMD), cast bf16.
  - xn transposed in 128-col chunks on the PE (transpose-mode matmul
    against an identity) -> xtn [d, b] tiles.
  - mm1: ips[b, k] = xtn_chunk.T @ C^T slice, PSUM f32. Two matmuls with
    the same tile_position share each [128, 512] PSUM bank so the exp
    that follows runs one [128, 512] ACT op (PSUM -> SBUF, bf16 out).
  - expT: PE transposes of exp chunks -> PSUM -> SBUF [k-chunk, 512 b]
    (exits split DVE/ACT), the stationary operand for mm2.
  - mm2: moving operand is [C[m, k-chunk, :] | 1] (65 cols): one matmul
    per (m, k-chunk) yields both y (reconstruction numerator) and the
    softmax denominator in PSUM [128, 4, 65] banks.
  - x_hat = y * recip(sums) (GPSIMD), codes = exp * recip(sums) (DVE),
    DMA out.

Precision: matmul operands and the stored exp are bf16 (PSUM accumulation
is fp32). End-to-end error vs the fp32 reference is ~1.8e-3 relative L2
(max abs err ~3e-3 of the output scale) on both outputs.
"""

from contextlib import ExitStack

import numpy as np

import concourse.bass as bass
import concourse.mybir as mybir
import concourse.tile as tile
from concourse.bass import ts
from concourse.bass_utils import run_bass_kernel_spmd
from concourse.masks import make_identity

F32 = mybir.dt.float32
P = 128
M, K, D = 8, 256, 64
FEAT = 512
NSUB = 4
BLK = P * NSUB
N_CORES = 8

CFG = dict(
    xs=4, nrm=4, sml=8, xn=8, xtn=8, expp=24, expt=24, cst=4, yx=4, rs=8,
    tp_ps=2, ips_ps=2, y_ps=4,
)


# --------------------------------------------------------------------------
# Walrus compatibility fixup: this toolchain's walrus encodes at most ONE
# sync-wait command per instruction and cannot encode `sem-eq-imm` (it
# needs two commands). Rewrite `== 0` waits to `<= 0` (semaphores are
# unsigned) and hoist extra waits onto standalone EventSemaphore
# instructions on the same engine (engines execute their stream in order,
# so waiting earlier is conservative and safe).
# --------------------------------------------------------------------------
def _fix_walrus_compat(nc):
    uid = 0
    for f in nc.m.functions:
        for bb in f.blocks:
            insts = bb.instructions
            i = 0
            while i < len(insts):
                inst = insts[i]
                si = inst.sync_info
                if si is None or not si.on_wait:
                    i += 1
                    continue
                waits = list(si.on_wait)
                changed = False
                for j, w in enumerate(waits):
                    if w.wait_mode == "sem-eq-imm":
                        assert w.wait_value == 0
                        waits[j] = mybir.SyncWait(
                            sync_type=w.sync_type, id=w.id,
                            ant_name=w.ant_name, wait_mode="sem-le-imm",
                            wait_value=0, wait_reg=w.wait_reg)
                        changed = True
                if len(waits) > 1:
                    for w in waits[:-1]:
                        ev = mybir.InstEventSemaphore(
                            name=f"I-wfix-{id(bb) & 0xFFFF}-{uid}",
                            ins=[], outs=[])
                        uid += 1
                        ev.engine = inst.engine
                        ev.sync_info = mybir.SyncInfo(on_wait=[w], on_update=[])
                        insts.insert(i, ev)
                        i += 1
                    waits = waits[-1:]
                    changed = True
                if changed:
                    inst.sync_info = mybir.SyncInfo(
                        on_wait=waits, on_update=list(si.on_update))
                i += 1
    return nc


def build_pq(bs: int, use_bf16: bool = True, nblk: int | None = None,
             nrep: int = 1, cfg=CFG):
    DT = mybir.dt.bfloat16 if use_bf16 else F32
    n_blocks = bs // BLK if nblk is None else nblk

    nc = bass.Bass()
    x_in = nc.dram_tensor("x", [bs, FEAT], F32, kind="ExternalInput")
    ct2_in = nc.dram_tensor("ct2", [P, M * K], DT, kind="ExternalInput")
    ckdo_in = nc.dram_tensor("ckdo", [P, M * 2 * 65], DT, kind="ExternalInput")
    xhat_out = nc.dram_tensor("xhat", [bs, FEAT], F32, kind="ExternalOutput")
    codes_out = nc.dram_tensor("codes", [bs, M * K], F32, kind="ExternalOutput")

    with tile.TileContext(nc) as tc, ExitStack() as ctx:
        singles = ctx.enter_context(tc.tile_pool(name="singles", bufs=1))
        pools = {}
        for name in ["xs", "nrm", "sml", "xn", "xtn", "expp", "expt", "cst",
                     "yx", "rs"]:
            pools[name] = ctx.enter_context(
                tc.tile_pool(name=name, bufs=cfg[name]))
        tp_ps = ctx.enter_context(
            tc.tile_pool(name="tp_ps", bufs=cfg["tp_ps"], space="PSUM"))
        ips_ps = ctx.enter_context(
            tc.tile_pool(name="ips_ps", bufs=cfg["ips_ps"], space="PSUM"))
        y_ps = ctx.enter_context(
            tc.tile_pool(name="y_ps", bufs=cfg["y_ps"], space="PSUM"))

        ident = singles.tile([P, P], DT, tag="ident")
        make_identity(nc, ident)
        ct2 = singles.tile([P, M * K], DT, tag="ct2")
        nc.sync.dma_start(out=ct2, in_=ct2_in[:, :])
        ckdo = singles.tile([P, M * 2 * 65], DT, tag="ckdo")
        nc.sync.dma_start(out=ckdo, in_=ckdo_in[:, :])

        for _rep in range(nrep):
            for blk in range(n_blocks):
                r0 = blk * BLK
                # ------------- phase A: load + normalize ---------------
                xn_t = []
                for i in range(NSUB):
                    xs = pools["xs"].tile([P, FEAT], F32, tag="xs")
                    nc.sync.dma_start(
                        out=xs, in_=x_in[r0 + i * P: r0 + (i + 1) * P, :])
                    sq = pools["nrm"].tile([P, FEAT], F32, tag="sq")
                    nc.gpsimd.tensor_mul(sq, xs, xs)
                    ss = pools["sml"].tile([P, M], F32, tag="ss")
                    nc.vector.reduce_sum(
                        ss, sq.rearrange("p (g d) -> p g d", g=M),
                        axis=mybir.AxisListType.X)
                    ln = pools["sml"].tile([P, M], F32, tag="ln")
                    nc.scalar.activation(
                        ln, ss, mybir.ActivationFunctionType.Ln)
                    rn = pools["sml"].tile([P, M], F32, tag="rn")
                    nc.scalar.activation(
                        rn, ln, mybir.ActivationFunctionType.Exp, scale=-0.5)
                    xn = pools["xn"].tile([P, FEAT], DT, tag="xn")
                    for m in range(M):
                        nc.gpsimd.tensor_scalar_mul(
                            xn[:, ts(m, D)], xs[:, ts(m, D)], rn[:, m:m + 1])
                    xn_t.append(xn)
                # ------------- phase T: transpose xn -> xtn ------------
                xtn_t = []
                for j in range(NSUB):
                    tp = tp_ps.tile([P, BLK], DT, tag="tp")
                    for i in range(NSUB):
                        nc.tensor.transpose(
                            tp[:, ts(i, P)], xn_t[i][:, ts(j, P)], ident)
                    xtn = pools["xtn"].tile([P, BLK], DT, tag="xtn")
                    nc.vector.tensor_copy(xtn, tp)
                    xtn_t.append(xtn)
                # ------------- phase B: mm1 + exp ----------------------
                # ips bank (q, p, i) holds m = 2q+p (half 0) and
                # m = 2q+p+4 (half 1): both matmuls use base partition
                # 64*p -> same tile_position -> may share the bank.
                exp_t = {}   # (m, i) -> (tile, half)
                for q in range(2):
                    for p in range(2):
                        for i in range(NSUB):
                            ips = ips_ps.tile([P, 2 * K], F32, tag="ips")
                            for h in range(2):
                                m = 2 * q + p + 4 * h
                                u = m // 2
                                lo, hi = 64 * p, 64 * p + 64
                                nc.tensor.matmul(
                                    ips[:, ts(h, K)],
                                    xtn_t[u][lo:hi, ts(i, P)],
                                    ct2[lo:hi, ts(m, K)],
                                    start=True, stop=True,
                                    skip_group_check=True)
                            ex = pools["expp"].tile([P, 2 * K], DT, tag="ex")
                            nc.scalar.activation(
                                ex, ips, mybir.ActivationFunctionType.Exp)
                            for h in range(2):
                                m = 2 * q + p + 4 * h
                                exp_t[(m, i)] = (ex, h)
                # ------------- phase TE: transpose exp -> expT ---------
                expT_t = {}
                for m in range(M):
                    for kc in range(2):
                        tp = tp_ps.tile([P, BLK], DT, tag="tp")
                        for i in range(NSUB):
                            ex, h = exp_t[(m, i)]
                            nc.tensor.transpose(
                                tp[:, ts(i, P)],
                                ex[:, h * K + kc * P: h * K + (kc + 1) * P],
                                ident)
                        et = pools["expt"].tile([P, BLK], DT, tag="et")
                        if (2 * m + kc) % 3 == 0:
                            nc.scalar.copy(et, tp)
                        else:
                            nc.vector.tensor_copy(et, tp)
                        expT_t[(m, kc)] = et
                # ------------- phase C: mm2 (y | sums) -----------------
                for i in range(NSUB):
                    yq_a = y_ps.tile([P, 4, 65], F32, tag="yq")
                    yq_b = y_ps.tile([P, 4, 65], F32, tag="yq")
                    yq = [yq_a, yq_b]
                    for m in range(M):
                        for kc in range(2):
                            nc.tensor.matmul(
                                yq[m // 4][:, m % 4, :],
                                expT_t[(m, kc)][:, ts(i, P)],
                                ckdo[:, (m * 2 + kc) * 65:
                                     (m * 2 + kc) * 65 + 65],
                                start=(kc == 0), stop=(kc == 1),
                                skip_group_check=True)
                    yb = pools["yx"].tile([P, 2, 4, 65], F32, tag="yb")
                    for half in range(2):
                        nc.scalar.copy(
                            yb[:, half],
                            yq[half].rearrange("p m d -> p (m d)"))
                    rec = pools["rs"].tile([P, M], F32, tag="rec")
                    nc.vector.reciprocal(
                        rec, yb.rearrange("p h m d -> p (h m) d")[:, :, 64])
                    xh = pools["yx"].tile([P, FEAT], F32, tag="xh")
                    for m in range(M):
                        nc.gpsimd.tensor_scalar_mul(
                            xh[:, ts(m, D)],
                            yb[:, m // 4, m % 4, 0:64],
                            rec[:, m:m + 1])
                    nc.sync.dma_start(
                        out=xhat_out[r0 + i * P: r0 + (i + 1) * P, :], in_=xh)
                    cs = pools["cst"].tile([P, M * K], F32, tag="cs")
                    for m in range(M):
                        ex, h = exp_t[(m, i)]
                        nc.vector.tensor_scalar_mul(
                            cs[:, ts(m, K)], ex[:, ts(h, K)], rec[:, m:m + 1])
                    nc.sync.dma_start(
                        out=codes_out[r0 + i * P: r0 + (i + 1) * P, :], in_=cs)
    return nc


def host_constants(C: np.ndarray, use_bf16: bool = True):
    """ct2 [128, 8*256]: ct2[d(+64 for odd m), m*256+k] = C[m,k,d];
    ckdo [128, 8*2*65]: per (m, kc): [C[m, kc*128:+128, :] | ones]."""
    import ml_dtypes
    dt = ml_dtypes.bfloat16 if use_bf16 else np.float32
    ct2 = np.zeros((P, M * K), dtype=np.float32)
    for m in range(M):
        rows = slice(0, 64) if m % 2 == 0 else slice(64, 128)
        ct2[rows, m * K:(m + 1) * K] = C[m].T
    ckdo = np.zeros((P, M * 2 * 65), dtype=np.float32)
    for m in range(M):
        for kc in range(2):
            base = (m * 2 + kc) * 65
            ckdo[:, base: base + 64] = C[m, kc * P:(kc + 1) * P, :]
            ckdo[:, base + 64] = 1.0
    return ct2.astype(dt), ckdo.astype(dt)


_CACHE = {}


def kernel(x: np.ndarray, C: np.ndarray):
    """Full-input entry point. Shards over 8 NeuronCores, returns the
    FULL outputs as the reference does: (x_hat [B,512] f32,
    codes [B,8,256] f32)."""
    x = np.ascontiguousarray(np.asarray(x, dtype=np.float32))
    C = np.ascontiguousarray(np.asarray(C, dtype=np.float32))
    B = x.shape[0]
    assert x.shape == (B, FEAT) and C.shape == (M, K, D)
    assert B % (N_CORES * BLK) == 0, f"B={B} must divide {N_CORES * BLK}"
    bs = B // N_CORES

    key = ("pq", bs)
    if key not in _CACHE:
        nc = build_pq(bs, use_bf16=True)
        _fix_walrus_compat(nc)
        _CACHE[key] = nc
    nc = _CACHE[key]

    ct2, ckdo = host_constants(C, use_bf16=True)
    ct2 = np.asarray(ct2)
    ckdo = np.asarray(ckdo)
    in_maps = [
        {"x": x[c * bs:(c + 1) * bs], "ct2": ct2, "ckdo": ckdo}
        for c in range(N_CORES)
    ]
    res = run_bass_kernel_spmd(nc, in_maps, list(range(N_CORES)))
    x_hat = np.concatenate([res.results[c]["xhat"] for c in range(N_CORES)], 0)
    codes = np.concatenate(
        [res.results[c]["codes"] for c in range(N_CORES)], 0
    ).reshape(B, M, K)
    return x_hat, codes
